# revision 1
# baseline (speedup 1.0000x reference)
"""Trainium2 Bass kernel for nn_DevConv (gnn_message_passing, N=8192).

Math (reference): per node i,
  maxd2[i] = relu(max over {j: adj[i,j]>0} of ||w*(x_i-x_j)||^2)
  out[i]   = 0.5*(prev[i] + mean(W_phi)*sqrt(maxd2[i]))

Distribution: node dim sharded across 8 cores; each core owns a
[1024, 8192] slab of adjacency (the memory-bound input: 32 MiB/core).

Device pipeline per core: 9 i-tiles at row step 114 (112 for the last),
each DMA'd as a FULL [128, 8192] int32 block (128 partitions = full-rate
DMA; ~12.5% overlap overhead), split into 4 column-groups of 2048:
  SP  : stream adjacency tile blocks HBM->SBUF, double buffered
  ACT : cast adjacency int32 -> bf16 ({0,1} exact) into partitions
        0..113 of the combined rhs buffer; partitions 117..127 hold the
        constant y-rows (2-way bf16 split of -2*y and sq)
  PE  : ONE matmul per 512-chunk, K=128, constant weights per tile:
        lhsT = [BIG*I ; 0 ; y-lhs rows] so
        psum[i,j] = BIG*adj[i,j] + sq_j - 2*y_i.y_j
        (identical weights for 16 consecutive matmuls keep the PE warm
        with LDWEIGHTS hidden: ~208ns per 512-col matmul)
  DVE : reduce-max over each [M_t, 2048] psum group -> acc[:, G]
Host epilogue (O(N)): fold groups/tiles, add sq_i - BIG, relu, sqrt,
scale by mean(W_phi), combine with prev.

The BIG-offset mask is exact: BIG > max possible d2, so rows with a
neighbor give BIG + max_nb d2; rows without stay < BIG and the final
relu clamps them to 0 (matching where(...,-inf) + max(,0)).
"""
from contextlib import ExitStack

import numpy as np
import ml_dtypes

import concourse.bacc as bacc
from concourse import mybir
from concourse.bass_utils import run_bass_kernel_spmd

BF16 = ml_dtypes.bfloat16

N = 8192
CORES = 8
ROWS = N // CORES            # 1024 rows per core
STEP = 114                   # i-rows advanced per tile
TILES = 9                    # 8 x 114 + 112 = 1024
K_Y = 11                     # y contraction rows (2-way split)
Y_P0 = 117                   # partition where y-rows live (117..127)
GROUP_W = 2048               # columns per psum group (4 banks)
GROUPS = N // GROUP_W        # 4 groups per tile
NG = TILES * GROUPS          # 36 groups per core
CHUNK = 512                  # matmul free dim (1 psum bank fp32)

_NC = {}


def _tile_rows(t):
    return STEP if t < TILES - 1 else ROWS - STEP * (TILES - 1)


def _build_nc(reps=1, stage="full"):
    """Build the per-core program. reps>1 replays the whole pipeline on the
    same inputs (for HW-time measurement via wall-clock deltas).
    stage in {dma, act, pe, full, peraw}: pipeline prefix, for bisection."""
    if (reps, stage) in _NC:
        return _NC[(reps, stage)]
    nc = bacc.Bacc("TRN2", target_bir_lowering=False, debug=False, num_devices=CORES)

    adj_d = nc.declare_dram_parameter("adj", [ROWS + 128, N], mybir.dt.int32, isOutput=False)
    lhsT_d = nc.declare_dram_parameter(
        "lhsT", [128, TILES * STEP + 16], mybir.dt.bfloat16, isOutput=False
    )
    yT_d = nc.declare_dram_parameter("yT", [K_Y, N], mybir.dt.bfloat16, isOutput=False)
    gmax_d = nc.declare_dram_parameter("gmax", [128, NG], mybir.dt.float32, isOutput=True)

    adj_sb = [nc.alloc_sbuf_tensor(f"adjsb{i}", [128, N], mybir.dt.int32) for i in range(3)]
    # combined rhs: partitions 0..113 <- cast adjacency, 117..127 <- y rows
    bf_sb = [nc.alloc_sbuf_tensor(f"bfsb{i}", [128, N], mybir.dt.bfloat16) for i in range(2)]
    lhsT_sb = nc.alloc_sbuf_tensor("lhsTsb", [128, TILES * STEP + 16], mybir.dt.bfloat16)
    acc_sb = nc.alloc_sbuf_tensor("accsb", [128, NG], mybir.dt.float32)
    ps = [nc.alloc_psum_tensor(f"ps{i}", [128, GROUP_W], mybir.dt.float32) for i in range(2)]

    with ExitStack() as es:
        block = es.enter_context(nc.Block())
        const_sem = es.enter_context(nc.semaphore("const_sem"))
        a_sems = [es.enter_context(nc.semaphore(f"a_sem{t}")) for t in range(TILES)]
        act_sem = es.enter_context(nc.semaphore("act_sem"))
        pe_sem = es.enter_context(nc.semaphore("pe_sem"))
        dve_sem = es.enter_context(nc.semaphore("dve_sem"))
        out_sem = es.enter_context(nc.semaphore("out_sem"))

        NT = TILES * reps  # global tile count across reps
        has_dma = stage != "peraw"
        has_act = stage in ("act", "pe", "full")
        has_pe = stage in ("pe", "full", "peraw")
        has_dve = stage == "full"

        @block.sync
        def _(sp):
            sp.dma_start(out=lhsT_sb[:, :], in_=lhsT_d[:, :]).then_inc(const_sem, 16)
            for b in range(2):
                sp.dma_start(
                    out=bf_sb[b][Y_P0 : Y_P0 + K_Y, :], in_=yT_d[:, :]
                ).then_inc(const_sem, 16)
            if has_dma:
                for T in range(NT):
                    t = T % TILES
                    if T >= 3 and has_act:
                        # adjacency slot T%3 is free once ACT cast tile T-3
                        sp.wait_ge(act_sem, GROUPS * (T - 3) + GROUPS)
                    # always a full 128-row block (full-rate DMA); the
                    # slab is padded host-side so the last tile stays aligned
                    lo = t * STEP
                    sp.dma_start(
                        out=adj_sb[T % 3][:, :], in_=adj_d[lo : lo + 128, :]
                    ).then_inc(a_sems[t], 16)
            if has_dve:
                sp.wait_ge(dve_sem, NG * reps)
            elif has_pe:
                sp.wait_ge(pe_sem, NG * reps)
            elif has_act:
                sp.wait_ge(act_sem, NG * reps)
            else:
                for t in range(TILES):
                    sp.wait_ge(a_sems[t], 16 * reps)
            sp.dma_start(out=gmax_d[:, :], in_=acc_sb[:, :]).then_inc(out_sem, 16)
            sp.wait_ge(out_sem, 16)

        if has_act:

            @block.scalar
            def _(act):
                for T in range(NT):
                    t = T % TILES
                    mt = _tile_rows(t)
                    act.wait_ge(a_sems[t], 16 * (T // TILES + 1))
                    if T >= 2 and has_pe:
                        # bf16 slot T%2 is free once PE consumed tile T-2
                        act.wait_ge(pe_sem, GROUPS * (T - 2) + GROUPS)
                    for g in range(GROUPS):
                        sl = slice(g * GROUP_W, (g + 1) * GROUP_W)
                        # cast Y_P0 (117) rows, not mt: rows mt..116 are
                        # zero-weighted in lhsT but must be finite (0*NaN=NaN)
                        act.activation(
                            out=bf_sb[T % 2][0:Y_P0, sl],
                            in_=adj_sb[T % 3][0:Y_P0, sl],
                            func=mybir.ActivationFunctionType.Copy,
                        ).then_inc(act_sem)

        if has_pe:

            @block.tensor
            def _(pe):
                pe.wait_ge(const_sem, 48)
                for T in range(NT):
                    t = T % TILES
                    mt = _tile_rows(t)
                    lhsT = lhsT_sb[:, t * STEP : t * STEP + mt]
                    for g in range(GROUPS):
                        G = T * GROUPS + g
                        if has_act:
                            pe.wait_ge(act_sem, G + 1)
                        if G >= 2 and has_dve:
                            # psum buffer G%2 is free once DVE reduced group G-2
                            pe.wait_ge(dve_sem, G - 1)
                        base = g * GROUP_W
                        mm = None
                        for c in range(GROUP_W // CHUNK):
                            mm = pe.matmul(
                                ps[G % 2][0:mt, c * CHUNK : (c + 1) * CHUNK],
                                lhsT,
                                bf_sb[T % 2][:, base + c * CHUNK : base + (c + 1) * CHUNK],
                                start=True,
                                stop=True,
                            )
                        mm.then_inc(pe_sem)

        if has_dve:

            @block.vector
            def _(dve):
                for G in range(NG * reps):
                    t = (G // GROUPS) % TILES
                    mt = _tile_rows(t)
                    dve.wait_ge(pe_sem, G + 1)
                    dve.tensor_reduce(
                        out=acc_sb[0:mt, G % NG : G % NG + 1],
                        in_=ps[G % 2][0:mt, :],
                        axis=mybir.AxisListType.X,
                        op=mybir.AluOpType.max,
                    ).then_inc(dve_sem)

    nc.compile()
    _NC[(reps, stage)] = nc
    return nc


def _split2(v):
    """2-way bf16 split: v ~= h + l with ~2^-16 rel residual."""
    h = v.astype(BF16)
    l = (v - h.astype(np.float32)).astype(BF16)
    return h, l


def _build_rows(y, sq):
    """y-side lhs rows [11, N] (columns = node i, already * -2) and rhs
    rows [11, N] (columns = j): sum_k lhs[k,i]*rhs[k,j] = sq_j - 2 y_i.y_j
    (up to ~2^-16 relative from the dropped l*l products)."""
    n = y.shape[0]
    bh, bl = _split2(y)
    b = {"h": bh, "l": bl}
    sh, sl = _split2(sq)
    ones = np.ones(n, dtype=BF16)

    pairs = [("h", "h"), ("h", "l"), ("l", "h")]
    lhs_rows, rhs_rows = [], []
    for c in range(3):
        for p1, p2 in pairs:
            lhs_rows.append((-2.0 * b[p1][:, c].astype(np.float32)).astype(BF16))
            rhs_rows.append(b[p2][:, c])
    for s_part in (sh, sl):
        lhs_rows.append(ones)
        rhs_rows.append(s_part)
    return np.stack(lhs_rows, axis=0), np.stack(rhs_rows, axis=0)


def _prepare(previous_inclusion_score, nodes, adjacency_matrix, W_phi, W_theta):
    prev = np.asarray(previous_inclusion_score, dtype=np.float32)
    nodes = np.asarray(nodes, dtype=np.float32)
    adj = np.ascontiguousarray(np.asarray(adjacency_matrix, dtype=np.int32))
    W_phi = np.asarray(W_phi, dtype=np.float32)
    w = np.asarray(W_theta, dtype=np.float32)[:, 0]

    y = (nodes * w[None, :]).astype(np.float32)
    sq = np.sum(y * y, axis=1, dtype=np.float32)

    # BIG: power of two strictly above any possible d2 = ||y_i - y_j||^2
    bound = 4.0 * float(sq.max()) + 8.0
    BIG = np.float32(2.0 ** int(np.ceil(np.log2(bound))))

    ylhs, yT = _build_rows(y, sq)  # [11, N] bf16 each
    eye = np.eye(128, dtype=np.float32) * BIG

    # per-core slab views padded to ROWS+128 rows so every tile DMA is a
    # full-rate [128, N] block; pad rows are ignored by the compute
    pad_last = np.concatenate([adj[(CORES - 1) * ROWS :], adj[:128]], axis=0)
    in_maps = []
    for k in range(CORES):
        adj_k = adj[k * ROWS : k * ROWS + ROWS + 128] if k < CORES - 1 else pad_last
        # lhsT_all [128, 9*114+16]: per tile t at column offset t*STEP:
        #   rows 0..mt-1   = BIG * I[:, :mt]
        #   rows mt..116   = 0
        #   rows 117..127  = y-lhs rows for this tile's nodes
        lhsT_all = np.zeros((128, TILES * STEP + 16), dtype=BF16)
        for t in range(TILES):
            mt = _tile_rows(t)
            cols = slice(t * STEP, t * STEP + mt)
            lhsT_all[0:mt, cols] = eye[0:mt, 0:mt].astype(BF16)
            node_lo = k * ROWS + t * STEP
            lhsT_all[Y_P0:128, cols] = ylhs[:, node_lo : node_lo + mt]
        in_maps.append({"adj": adj_k, "lhsT": lhsT_all, "yT": yT})
    return in_maps, prev, sq, BIG, W_phi


def _finish(res, prev, sq, BIG, W_phi):
    m = np.empty(N, dtype=np.float32)
    for k in range(CORES):
        gm = res.results[k]["gmax"]                      # [128, 36] (p, t*4+g)
        tm = gm.reshape(128, TILES, GROUPS).max(axis=2)  # [128, 9]
        for t in range(TILES):
            mt = _tile_rows(t)
            lo = k * ROWS + t * STEP
            m[lo : lo + mt] = tm[0:mt, t]

    maxd2 = np.maximum(m + sq - BIG, 0.0)
    max_dist = np.sqrt(maxd2)
    inc_mean = (max_dist[:, None] * W_phi[None, :]).mean(axis=1).astype(np.float32)
    return ((prev + inc_mean) * 0.5).astype(np.float32)


def kernel(previous_inclusion_score, nodes, adjacency_matrix, W_phi, W_theta):
    in_maps, prev, sq, BIG, W_phi = _prepare(
        previous_inclusion_score, nodes, adjacency_matrix, W_phi, W_theta
    )
    nc = _build_nc()
    res = run_bass_kernel_spmd(nc, in_maps, list(range(CORES)))
    return _finish(res, prev, sq, BIG, W_phi)



# revision 22
# speedup vs baseline: 1.4065x; 1.4065x over previous
"""Trainium2 Bass kernel for nn_DevConv (gnn_message_passing, N=8192).

Math (reference): per node i,
  maxd2[i] = relu(max over {j: adj[i,j]>0} of ||w*(x_i-x_j)||^2)
  out[i]   = 0.5*(prev[i] + mean(W_phi)*sqrt(maxd2[i]))

Distribution: node dim sharded across 8 cores; each core owns a
[1024, 8192] slab of adjacency, host-cast to fp8 e4m3 ({0,1} exact) so the
memory-bound input is 8 MiB/core instead of 32.

Device pipeline per core, 8 i-tiles of 128 rows:
  SP  : stream adjacency tiles HBM->SBUF fp8, double buffered
  PE  : fp8 DoubleRow matmuls, one per 512-col chunk.  DoubleRow contracts
        two "planes" of K=128 each at 0.5 cycles/out-col:
          plane A: lhsT = BIG*I[128],  rhs = adjacency tile rows
          plane B: lhsT = y-lhs rows,  rhs = constant yT rows (+ zeros)
        so psum[i,j] = s*(BIG0*adj[i,j] + sq_j - 2*y_i.y_j).  The y side is
        a 3-way fp8 split (18 product rows + 3 sq rows = 21 K-rows,
        constant, DMA'd once) giving ~2^-11 relative d2 error.
  Reduction (PSUM has a 1-port read limit per instruction, and only DVE
  and ACT can read it, 1 elem/cycle/lane each; TensorTensorReduce is
  broken on this runtime, so only copy/stt/tt/reduce are used):
    ACT : copies psum groups to SBUF bf16 with a fused -128 bias
          (removes the mask offset so bf16 keeps ~8 bits on d2 itself)
    DVE : scalar_tensor_tensor (psum_group - 128) max copied_group ->
          bf16, ingesting 1 psum + 1 sbuf elem per cycle; extra copies
          fold pairwise at 4x (packed bf16); per tile a short 4x fold
          chain + one [128,256] reduce produces the tile's accum column.
  Tile types over each 8-tile rep: 5x A (2 copies + 2 psum-stt) and
  3x B (3 copies + 1 psum-stt) to balance ACT vs DVE ingest.
Host epilogue (O(N)): maxd2 = relu((m+128)/s + sq_i - BIG0), out =
0.5*(prev + mean(W_phi)*sqrt(maxd2)).
"""
from contextlib import ExitStack

import numpy as np
import ml_dtypes

import concourse.bacc as bacc
from concourse import mybir
from concourse.bass_utils import run_bass_kernel_spmd

F8 = ml_dtypes.float8_e4m3
F8_ONE = np.uint8(0x38)          # 1.0 in e4m3

N = 8192
CORES = 8
ROWS = N // CORES                # 1024 rows per core
TILES = 8                        # [128, N] i-tiles per core
TSTEP = 128
GROUP_W = 2048                   # columns per psum group (4 banks)
GROUPS = 4                       # psum groups per tile
NACC = TILES                     # one accum column per tile
CHUNK = 512                      # matmul free dim (1 psum bank fp32)
Y_ROWS = 21                      # y contraction rows (3-way fp8 split)
BIGV = 128.0                     # s*BIG0: the mask offset in psum units
NSLOT = 6                        # SBUF copy slots

# tile type by position in the 8-tile rep: True = type B (3 copies,
# 1 psum-stt), False = type A (2 copies, 2 psum-stt)
TYPE_B = [False, True, False, False, True, False, False, True]

_NC = {}


def _schedule():
    """Static per-rep schedule: for each tile, its copy groups / stt groups,
    plus running counters.  Returns per-tile dicts."""
    tiles = []
    k = 0   # copies
    q = 0   # psum-stts
    for t in range(TILES):
        if TYPE_B[t]:
            cg = [0, 1, 2]
            pg = [3]
        else:
            cg = [0, 1]
            pg = [2, 3]
        tiles.append(
            {
                "cg": cg,
                "pg": pg,
                "k0": k,                 # first copy index of this tile
                "q0": q,                 # first psum-stt index of this tile
                "copies": len(cg),
                "stts": len(pg),
            }
        )
        k += len(cg)
        q += len(pg)
    return tiles, k, q


SCHED, COPIES_PER_REP, STTS_PER_REP = _schedule()


DVE_PER_TILE = 7                      # stt_a, stt_b, 4 folds, reduce
DVE_PER_REP = TILES * DVE_PER_TILE


def _consumer_of(G):
    """(engine, completed_count) once group G (within-rep index) has been
    consumed: 'act' -> act_sem value, 'dve' -> dch_sem value (the DVE
    stream is fully ordered by dch_sem, one inc per instruction)."""
    T, g = divmod(G, GROUPS)
    info = SCHED[T]
    if g in info["cg"]:
        return "act", info["k0"] + info["cg"].index(g) + 1
    # psum-stt: stt_a at DVE index 7T, stt_b at 7T+1
    return "dve", DVE_PER_TILE * T + info["pg"].index(g) + 1


def _copy_consumer_dch(k):
    """dch_sem value once copy k (within-rep index) has been consumed."""
    for t in range(TILES):
        info = SCHED[t]
        if info["k0"] <= k < info["k0"] + info["copies"]:
            pos = k - info["k0"]
            if TYPE_B[t]:
                # cg[0] -> stt_a (7t), cg[1] and cg[2] -> stt_b (7t+1)
                idx = DVE_PER_TILE * t + (0 if pos == 0 else 1)
            else:
                idx = DVE_PER_TILE * t + pos
            return idx + 1
    raise AssertionError(k)


def _through_tile(T):
    """(copies, psum-stts) completed once tiles 0..T are fully consumed."""
    info = SCHED[T]
    return info["k0"] + info["copies"], info["q0"] + info["stts"]


def _build_nc(reps=1):
    """Build the per-core program. reps>1 replays the whole pipeline on the
    same inputs (for HW-time measurement via wall-clock deltas)."""
    if reps in _NC:
        return _NC[reps]
    nc = bacc.Bacc("TRN2", target_bir_lowering=False, debug=False, num_devices=CORES)

    adj_d = nc.declare_dram_parameter("adj", [ROWS, N], mybir.dt.float8e4, isOutput=False)
    lhsT_d = nc.declare_dram_parameter(
        "lhsT", [128, TILES * 2, TSTEP], mybir.dt.float8e4, isOutput=False
    )
    yz_d = nc.declare_dram_parameter("yz", [128, 1, N], mybir.dt.float8e4, isOutput=False)
    gmax_d = nc.declare_dram_parameter("gmax", [128, NACC], mybir.dt.float32, isOutput=True)

    BF = mybir.dt.bfloat16
    # rhs planes: 0 = adjacency buffer 0, 1 = constant yT rows + zeros,
    # 2 = adjacency buffer 1.  Tile T uses planes (0,1) or (1,2) so the
    # DoubleRow rhs AP is a contiguous 2-plane slice either way.
    rhs_all = nc.alloc_sbuf_tensor("rhs", [128, 3, N], mybir.dt.float8e4)
    lhsT_sb = nc.alloc_sbuf_tensor("lhsTsb", [128, TILES * 2, TSTEP], mybir.dt.float8e4)
    acc_sb = nc.alloc_sbuf_tensor("accsb", [128, NACC], mybir.dt.float32)
    cp = [nc.alloc_sbuf_tensor(f"cp{i}", [128, GROUP_W], BF) for i in range(NSLOT)]
    mA = [nc.alloc_sbuf_tensor(f"mA{i}", [128, GROUP_W], BF) for i in range(2)]
    mB = [nc.alloc_sbuf_tensor(f"mB{i}", [128, GROUP_W], BF) for i in range(2)]
    nf = [nc.alloc_sbuf_tensor(f"nf{i}", [128, GROUP_W], BF) for i in range(2)]
    n1 = [nc.alloc_sbuf_tensor(f"n1{i}", [128, GROUP_W // 2], BF) for i in range(2)]
    n2 = [nc.alloc_sbuf_tensor(f"n2{i}", [128, GROUP_W // 4], BF) for i in range(2)]
    n3 = [nc.alloc_sbuf_tensor(f"n3{i}", [128, GROUP_W // 8], BF) for i in range(2)]
    ps = [nc.alloc_psum_tensor(f"ps{i}", [128, GROUP_W], mybir.dt.float32) for i in range(2)]

    NT = TILES * reps
    MAX = mybir.AluOpType.max
    SUB = mybir.AluOpType.subtract
    BYP = mybir.AluOpType.bypass

    with ExitStack() as es:
        block = es.enter_context(nc.Block())
        const_sem = es.enter_context(nc.semaphore("const_sem"))
        yz_sem = es.enter_context(nc.semaphore("yz_sem"))
        a_sems = [es.enter_context(nc.semaphore(f"a_sem{t}")) for t in range(TILES)]
        pe_sem = es.enter_context(nc.semaphore("pe_sem"))
        act_sem = es.enter_context(nc.semaphore("act_sem"))
        dch_sem = es.enter_context(nc.semaphore("dch_sem"))
        out_sem = es.enter_context(nc.semaphore("out_sem"))

        @block.sync
        def _(sp):
            for T in range(NT):
                t = T % TILES
                if T >= 2:
                    r = (T - 2) // TILES
                    tp = (T - 2) % TILES
                    ck, _ = _through_tile(tp)
                    sp.wait_ge(act_sem, r * COPIES_PER_REP + ck)
                    # last psum-stt of tile tp: stt_b (A) at 7tp+1, stt_a (B)
                    last = DVE_PER_TILE * tp + (0 if TYPE_B[tp] else 1)
                    sp.wait_ge(dch_sem, r * DVE_PER_REP + last + 1)
                pl = 0 if T % 2 == 0 else 2
                sp.dma_start(
                    out=rhs_all[:, pl : pl + 1, :],
                    in_=adj_d[t * TSTEP : (t + 1) * TSTEP, :],
                ).then_inc(a_sems[t], 16)
            sp.wait_ge(dch_sem, DVE_PER_REP * reps)
            sp.dma_start(out=gmax_d[:, :], in_=acc_sb[:, :]).then_inc(out_sem, 16)
            sp.wait_ge(out_sem, 16)

        @block.scalar
        def _(act):
            # constants go out on ACT's DGE ring so they don't serialize
            # with the adjacency stream on SP's ring
            act.dma_start(out=lhsT_sb[:, :, :], in_=lhsT_d[:, :, :]).then_inc(
                const_sem, 16
            )
            act.dma_start(out=rhs_all[:, 1:2, :], in_=yz_d[:, :, :]).then_inc(
                yz_sem, 16
            )
            k = 0
            for R in range(reps):
                for t in range(TILES):
                    info = SCHED[t]
                    T = R * TILES + t
                    for g in info["cg"]:
                        G = T * GROUPS + g
                        act.wait_ge(pe_sem, G + 1)
                        if k >= NSLOT:
                            kp, kk = divmod(k - NSLOT, COPIES_PER_REP)
                            act.wait_ge(
                                dch_sem, kp * DVE_PER_REP + _copy_consumer_dch(kk)
                            )
                        act.activation(
                            out=cp[k % NSLOT][:, :],
                            in_=ps[G % 2][:, :],
                            func=mybir.ActivationFunctionType.Copy,
                            bias=-BIGV,
                        ).then_inc(act_sem)
                        k += 1

        @block.tensor
        def _(pe):
            pe.wait_ge(const_sem, 16)
            pe.wait_ge(yz_sem, 16)
            for T in range(NT):
                t = T % TILES
                lhsT = lhsT_sb[:, 2 * t : 2 * t + 2, :]
                pl = 0 if T % 2 == 0 else 1
                pe.wait_ge(a_sems[t], 16 * (T // TILES + 1))
                for g in range(GROUPS):
                    G = T * GROUPS + g
                    if G >= 2:
                        # psum buffer G%2 free once its consumer ran
                        pr, pg = divmod(G - 2, GROUPS * TILES)
                        eng, cnt = _consumer_of(pg)
                        if eng == "act":
                            pe.wait_ge(act_sem, pr * COPIES_PER_REP + cnt)
                        else:
                            pe.wait_ge(dch_sem, pr * DVE_PER_REP + cnt)
                    base = g * GROUP_W
                    mm = None
                    for c in range(GROUP_W // CHUNK):
                        lo = base + c * CHUNK
                        mm = pe.matmul(
                            ps[G % 2][:, c * CHUNK : (c + 1) * CHUNK],
                            lhsT,
                            rhs_all[:, pl : pl + 2, lo : lo + CHUNK],
                            start=True,
                            stop=True,
                            perf_mode=mybir.MatmulPerfMode.DoubleRow,
                        )
                    mm.then_inc(pe_sem)

        @block.vector
        def _(dve):
            di = 0   # global DVE instruction index (chain sem)

            def chain(inst):
                nonlocal di
                inst.then_inc(dch_sem)
                di += 1
                return inst

            for R in range(reps):
                for t in range(TILES):
                    info = SCHED[t]
                    T = R * TILES + t
                    p = T % 2
                    k0 = R * COPIES_PER_REP + info["k0"]
                    if di:
                        dve.wait_ge(dch_sem, di)
                    if TYPE_B[t]:
                        G = T * GROUPS + 3
                        dve.wait_ge(pe_sem, G + 1)
                        dve.wait_ge(act_sem, k0 + 1)
                        chain(
                            dve.scalar_tensor_tensor(
                                out=mA[p][:, :], in0=ps[G % 2][:, :], scalar=BIGV,
                                in1=cp[k0 % NSLOT][:, :], op0=SUB, op1=MAX,
                            )
                        )
                        dve.wait_ge(dch_sem, di)
                        dve.wait_ge(act_sem, k0 + 3)
                        chain(
                            dve.scalar_tensor_tensor(
                                out=mB[p][:, :], in0=cp[(k0 + 1) % NSLOT][:, :],
                                scalar=0.0, in1=cp[(k0 + 2) % NSLOT][:, :],
                                op0=BYP, op1=MAX,
                            )
                        )
                    else:
                        G2 = T * GROUPS + 2
                        G3 = T * GROUPS + 3
                        dve.wait_ge(pe_sem, G2 + 1)
                        dve.wait_ge(act_sem, k0 + 1)
                        chain(
                            dve.scalar_tensor_tensor(
                                out=mA[p][:, :], in0=ps[G2 % 2][:, :], scalar=BIGV,
                                in1=cp[k0 % NSLOT][:, :], op0=SUB, op1=MAX,
                            )
                        )
                        dve.wait_ge(dch_sem, di)
                        dve.wait_ge(pe_sem, G3 + 1)
                        dve.wait_ge(act_sem, k0 + 2)
                        chain(
                            dve.scalar_tensor_tensor(
                                out=mB[p][:, :], in0=ps[G3 % 2][:, :], scalar=BIGV,
                                in1=cp[(k0 + 1) % NSLOT][:, :], op0=SUB, op1=MAX,
                            )
                        )
                    # 4x packed bf16 fold chain (TensorScalarPtr gets 4x_2p;
                    # plain TensorTensor would only get 2x) + one short reduce
                    def fold(dst, lo, hi):
                        dve.wait_ge(dch_sem, di)
                        chain(
                            dve.scalar_tensor_tensor(
                                out=dst, in0=lo, scalar=0.0, in1=hi,
                                op0=BYP, op1=MAX,
                            )
                        )

                    fold(nf[p][:, :], mA[p][:, :], mB[p][:, :])
                    fold(n1[p][:, :], nf[p][:, 0:1024], nf[p][:, 1024:2048])
                    fold(n2[p][:, :], n1[p][:, 0:512], n1[p][:, 512:1024])
                    fold(n3[p][:, :], n2[p][:, 0:256], n2[p][:, 256:512])
                    dve.wait_ge(dch_sem, di)
                    chain(
                        dve.tensor_reduce(
                            out=acc_sb[:, t : t + 1], in_=n3[p][:, :],
                            axis=mybir.AxisListType.X, op=MAX,
                        )
                    )

    nc.compile()
    _NC[reps] = nc
    return nc


def _split3(v):
    """3-way fp8 split: v ~= a + b + c with ~2^-12 rel residual."""
    a = v.astype(F8)
    r = v - a.astype(np.float32)
    b = r.astype(F8)
    c = (r - b.astype(np.float32)).astype(F8)
    return a, b, c


def _prepare(previous_inclusion_score, nodes, adjacency_matrix, W_phi, W_theta):
    prev = np.asarray(previous_inclusion_score, dtype=np.float32)
    nodes = np.asarray(nodes, dtype=np.float32)
    adj = np.asarray(adjacency_matrix)
    W_phi = np.asarray(W_phi, dtype=np.float32)
    w = np.asarray(W_theta, dtype=np.float32)[:, 0]

    y = (nodes * w[None, :]).astype(np.float32)
    sq = np.sum(y * y, axis=1, dtype=np.float32)

    sqmax = float(sq.max())
    bound = 4.0 * sqmax + 8.0
    BIG0 = float(2.0 ** np.ceil(np.log2(bound)))   # > any possible d2
    s = BIGV / BIG0                                # psum scale, power of 2
    # keep sq rhs rows within e4m3 finite range (240)
    kq = int(np.ceil(np.log2(sqmax / 224.0))) if sqmax > 224.0 else 0

    adj_f8 = np.where(adj != 0, F8_ONE, np.uint8(0)).view(F8)

    # y rows: 18 product rows (-2*s*part_i x part_j per coord, pairs
    # aa ab ba ac ca bb) + 3 sq rows (lhs = s*2^kq, rhs = sq*2^-kq parts)
    lhs_rows = np.zeros((Y_ROWS, N), np.float32)
    rhs_rows = np.zeros((Y_ROWS, N), F8)
    r = 0
    for cdim in range(3):
        a, b, c = _split3(y[:, cdim])
        af, bf, cf = (x.astype(np.float32) for x in (a, b, c))
        for lv, rv in ((af, a), (af, b), (bf, a), (af, c), (cf, a), (bf, b)):
            lhs_rows[r] = -2.0 * s * lv
            rhs_rows[r] = rv
            r += 1
    q1, q2, q3 = _split3(sq * np.float32(2.0 ** -kq))
    for q in (q1, q2, q3):
        lhs_rows[r] = s * (2.0 ** kq)
        rhs_rows[r] = q
        r += 1
    assert r == Y_ROWS

    yz = np.zeros((128, 1, N), F8)
    yz[:Y_ROWS, 0, :] = rhs_rows

    eyeBIG = (np.eye(TSTEP, dtype=np.float32) * BIGV).astype(F8)
    lhs_f8 = lhs_rows.astype(F8)    # exact: power-of-2 scales of fp8 values

    in_maps = []
    for k in range(CORES):
        # per tile: two K=128 planes.  Even tiles use rhs planes (0,1) =
        # (adj, yT); odd tiles use (1,2) = (yT, adj).  Match that order.
        lhsT = np.zeros((128, TILES * 2, TSTEP), F8)
        for t in range(TILES):
            lo = k * ROWS + t * TSTEP
            adj_plane = 2 * t if t % 2 == 0 else 2 * t + 1
            y_plane = 2 * t + 1 if t % 2 == 0 else 2 * t
            lhsT[:, adj_plane, :] = eyeBIG
            lhsT[:Y_ROWS, y_plane, :] = lhs_f8[:, lo : lo + TSTEP]
        in_maps.append(
            {
                "adj": np.ascontiguousarray(adj_f8[k * ROWS : (k + 1) * ROWS]),
                "lhsT": lhsT,
                "yz": yz,
            }
        )
    return in_maps, prev, sq, BIG0, s, W_phi


def _finish(res, prev, sq, BIG0, s, W_phi):
    m = np.empty(N, dtype=np.float32)
    for k in range(CORES):
        gm = res.results[k]["gmax"]                  # [128, 8] (p, t)
        for t in range(TILES):
            lo = k * ROWS + t * TSTEP
            m[lo : lo + TSTEP] = gm[:, t]

    maxd2 = np.maximum(
        (m + np.float32(BIGV)) / np.float32(s) + (sq - np.float32(BIG0)), 0.0
    )
    max_dist = np.sqrt(maxd2)
    inc_mean = max_dist * np.float32(W_phi.mean())
    return ((prev + inc_mean) * 0.5).astype(np.float32)


def kernel(previous_inclusion_score, nodes, adjacency_matrix, W_phi, W_theta):
    in_maps, prev, sq, BIG0, s, W_phi = _prepare(
        previous_inclusion_score, nodes, adjacency_matrix, W_phi, W_theta
    )
    nc = _build_nc()
    res = run_bass_kernel_spmd(nc, in_maps, list(range(CORES)))
    return _finish(res, prev, sq, BIG0, s, W_phi)


# revision 27
# speedup vs baseline: 1.7471x; 1.2421x over previous
"""Trainium2 Bass kernel for nn_DevConv (gnn_message_passing, N=8192).

Math (reference): per node i,
  maxd2[i] = relu(max over {j: adj[i,j]>0} of ||w*(x_i-x_j)||^2)
  out[i]   = 0.5*(prev[i] + mean(W_phi)*sqrt(maxd2[i]))

Distribution: node dim sharded across 8 cores; each core owns a
[1024, 8192] slab of adjacency, host-cast to fp8 e4m3 ({0,1} exact) so the
memory-bound input is 8 MiB/core instead of 32.

Device pipeline per core, 8 i-tiles of 128 rows:
  SP  : stream adjacency tiles HBM->SBUF fp8, double buffered
  PE  : fp8 DoubleRow matmuls, one per 512-col chunk.  DoubleRow contracts
        two "planes" of K=128 each at 0.5 cycles/out-col:
          plane A: lhsT = BIG*I[128],  rhs = adjacency tile rows
          plane B: lhsT = y-lhs rows,  rhs = constant yT rows (+ zeros)
        so psum[i,j] = s*(BIG0*adj[i,j] + sq_j - 2*y_i.y_j).  The y side is
        a 3-way fp8 split (18 product rows + 3 sq rows = 21 K-rows,
        constant, DMA'd once) giving ~2^-11 relative d2 error.
  Reduction: PSUM allows one read port per instruction and only DVE/ACT
  can touch it (1 elem/cycle/lane each; TensorTensorReduce is broken on
  this runtime).  PSUM is one [128,4096] tensor used as a ring of four
  1024-wide groups (8 groups per tile) so PE runs 4 groups ahead of the
  consumers.  Per tile, uniformly:
    ACT : copies groups 0..4 to SBUF bf16 with a fused -128 bias
          (removes the mask offset so bf16 keeps ~8 bits on d2 itself)
    DVE : m_i = (psum group 5+i  - 128) max copy_i, i=0..2
          (scalar_tensor_tensor: 1 psum + 1 sbuf elem per cycle)
    POOL: m3 = copy_3 max copy_4, then f0 = m0 max m1, f1 = m2 max m3
          (InstTensorTensor from the gpsimd standard library)
    DVE : n1 = f0 max f1 (packed bf16, 2x), fold to 256 wide, one
          tensor_reduce -> the tile's accum column
Host epilogue (O(N)): maxd2 = relu((m+128)/s + sq_i - BIG0), out =
0.5*(prev + mean(W_phi)*sqrt(maxd2)).
"""
from contextlib import ExitStack

import numpy as np
import ml_dtypes

import concourse.bacc as bacc
from concourse import mybir
from concourse.bass_utils import run_bass_kernel_spmd

F8 = ml_dtypes.float8_e4m3
F8_ONE = np.uint8(0x38)          # 1.0 in e4m3

N = 8192
CORES = 8
ROWS = N // CORES                # 1024 rows per core
TILES = 8                        # [128, N] i-tiles per core
TSTEP = 128
GROUP_W = 1024                   # columns per psum group (2 banks)
GROUPS = 8                       # psum groups per tile
RING = 4                         # psum ring depth (4 x 1024 fp32 = 16KB)
NACC = TILES                     # one accum column per tile
CHUNK = 512                      # matmul free dim (1 psum bank fp32)
Y_ROWS = 21                      # y contraction rows (3-way fp8 split)
BIGV = 128.0                     # s*BIG0: the mask offset in psum units
NSLOT = 8                        # SBUF copy slots

NCOPY = 5                        # ACT copies per tile (groups 0..4)
NSTT = 3                         # DVE psum-stts per tile (groups 5..7)
COPIES_PER_REP = TILES * NCOPY   # 40
DVE_PER_TILE = NSTT + 7          # stts + bfold + f0 + f1 + n1..n3 + reduce
DVE_PER_REP = TILES * DVE_PER_TILE

_NC = {}


def _build_nc(reps=1):
    """Build the per-core program. reps>1 replays the whole pipeline on the
    same inputs (for HW-time measurement via wall-clock deltas)."""
    if reps in _NC:
        return _NC[reps]
    nc = bacc.Bacc("TRN2", target_bir_lowering=False, debug=False, num_devices=CORES)

    adj_d = nc.declare_dram_parameter("adj", [ROWS, N], mybir.dt.float8e4, isOutput=False)
    lhsT_d = nc.declare_dram_parameter(
        "lhsT", [128, TILES * 2, TSTEP], mybir.dt.float8e4, isOutput=False
    )
    yz_d = nc.declare_dram_parameter("yz", [128, 1, N], mybir.dt.float8e4, isOutput=False)
    gmax_d = nc.declare_dram_parameter("gmax", [128, NACC], mybir.dt.float32, isOutput=True)

    BF = mybir.dt.bfloat16
    rhs_all = nc.alloc_sbuf_tensor("rhs", [128, 3, N], mybir.dt.float8e4)
    lhsT_sb = nc.alloc_sbuf_tensor("lhsTsb", [128, TILES * 2, TSTEP], mybir.dt.float8e4)
    acc_sb = nc.alloc_sbuf_tensor("accsb", [128, NACC], mybir.dt.float32)
    cp = [nc.alloc_sbuf_tensor(f"cp{i}", [128, GROUP_W], BF) for i in range(NSLOT)]
    m = [
        [nc.alloc_sbuf_tensor(f"m{i}_{p}", [128, GROUP_W], BF) for i in range(4)]
        for p in range(2)
    ]
    f = [
        [nc.alloc_sbuf_tensor(f"f{i}_{p}", [128, GROUP_W], BF) for i in range(2)]
        for p in range(2)
    ]
    n1 = [nc.alloc_sbuf_tensor(f"n1{p}", [128, GROUP_W], BF) for p in range(2)]
    n2 = [nc.alloc_sbuf_tensor(f"n2{p}", [128, GROUP_W // 2], BF) for p in range(2)]
    n3 = [nc.alloc_sbuf_tensor(f"n3{p}", [128, GROUP_W // 4], BF) for p in range(2)]
    ps = nc.alloc_psum_tensor("ps", [128, RING * GROUP_W], mybir.dt.float32)

    NT = TILES * reps
    MAX = mybir.AluOpType.max
    SUB = mybir.AluOpType.subtract

    def psg(G):
        r = G % RING
        return ps[:, r * GROUP_W : (r + 1) * GROUP_W]

    with ExitStack() as es:
        block = es.enter_context(nc.Block())
        const_sem = es.enter_context(nc.semaphore("const_sem"))
        yz_sem = es.enter_context(nc.semaphore("yz_sem"))
        a_sems = [es.enter_context(nc.semaphore(f"a_sem{t}")) for t in range(TILES)]
        pe_sem = es.enter_context(nc.semaphore("pe_sem"))
        act_sem = es.enter_context(nc.semaphore("act_sem"))
        dch_sem = es.enter_context(nc.semaphore("dch_sem"))
        out_sem = es.enter_context(nc.semaphore("out_sem"))

        # global (across reps) index helpers
        def dch_stt(T, i):          # value once stt i of tile T is done
            return T * DVE_PER_TILE + i + 1

        def dch_bfold(T):
            return T * DVE_PER_TILE + NSTT + 1

        def copy_consumer(kk):
            """global dch value once global copy kk is consumed."""
            T, pos = divmod(kk, NCOPY)
            if pos < NSTT:
                return dch_stt(T, pos)
            return dch_bfold(T)

        @block.sync
        def _(sp):
            sp.dma_start(out=lhsT_sb[:, :, :], in_=lhsT_d[:, :, :]).then_inc(
                const_sem, 16
            )
            sp.dma_start(out=rhs_all[:, 1:2, :], in_=yz_d[:, :, :]).then_inc(
                yz_sem, 16
            )
            for T in range(NT):
                t = T % TILES
                if T >= 2:
                    # rhs plane T%2 is only read by PE: free once all of
                    # tile T-2's matmul groups have issued
                    sp.wait_ge(pe_sem, GROUPS * (T - 1))
                pl = 0 if T % 2 == 0 else 2
                sp.dma_start(
                    out=rhs_all[:, pl : pl + 1, :],
                    in_=adj_d[t * TSTEP : (t + 1) * TSTEP, :],
                ).then_inc(a_sems[t], 16)
            sp.wait_ge(dch_sem, DVE_PER_REP * reps)
            sp.dma_start(out=gmax_d[:, :], in_=acc_sb[:, :]).then_inc(out_sem, 16)
            sp.wait_ge(out_sem, 16)

        @block.scalar
        def _(act):
            for k in range(COPIES_PER_REP * reps):
                T, pos = divmod(k, NCOPY)
                G = T * GROUPS + pos
                act.wait_ge(pe_sem, G + 1)
                if k >= NSLOT:
                    act.wait_ge(dch_sem, copy_consumer(k - NSLOT))
                act.activation(
                    out=cp[k % NSLOT][:, :],
                    in_=psg(G),
                    func=mybir.ActivationFunctionType.Copy,
                    bias=-BIGV,
                ).then_inc(act_sem)

        @block.tensor
        def _(pe):
            pe.wait_ge(const_sem, 16)
            pe.wait_ge(yz_sem, 16)
            for T in range(NT):
                t = T % TILES
                lhsT = lhsT_sb[:, 2 * t : 2 * t + 2, :]
                pl = 0 if T % 2 == 0 else 1
                pe.wait_ge(a_sems[t], 16 * (T // TILES + 1))
                for g in range(GROUPS):
                    G = T * GROUPS + g
                    if G >= RING:
                        Tp, pp = divmod(G - RING, GROUPS)
                        if pp < NCOPY:
                            pe.wait_ge(act_sem, Tp * NCOPY + pp + 1)
                        else:
                            pe.wait_ge(dch_sem, dch_stt(Tp, pp - NCOPY))
                    base = (G % RING) * GROUP_W
                    mm = None
                    for c in range(GROUP_W // CHUNK):
                        lo = g * GROUP_W + c * CHUNK
                        mm = pe.matmul(
                            ps[:, base + c * CHUNK : base + (c + 1) * CHUNK],
                            lhsT,
                            rhs_all[:, pl : pl + 2, lo : lo + CHUNK],
                            start=True,
                            stop=True,
                            perf_mode=mybir.MatmulPerfMode.DoubleRow,
                        )
                    mm.then_inc(pe_sem)

        @block.vector
        def _(dve):
            di = 0

            def chain(inst):
                nonlocal di
                inst.then_inc(dch_sem)
                di += 1
                return inst

            for T in range(NT):
                t = T % TILES
                p = T % 2
                k0 = T * NCOPY
                if di:
                    dve.wait_ge(dch_sem, di)
                for i in range(NSTT):
                    G = T * GROUPS + NCOPY + i
                    if i:
                        dve.wait_ge(dch_sem, di)
                    dve.wait_ge(pe_sem, G + 1)
                    dve.wait_ge(act_sem, k0 + i + 1)
                    chain(
                        dve.scalar_tensor_tensor(
                            out=m[p][i][:, :], in0=psg(G), scalar=BIGV,
                            in1=cp[(k0 + i) % NSLOT][:, :], op0=SUB, op1=MAX,
                        )
                    )
                # tail (all packed-bf16 2x on DVE; Pool can't run
                # vector ops in this toolchain): m3 = cp3 max cp4, two
                # level-1 folds, then n1 and the shrink chain
                dve.wait_ge(dch_sem, di)
                dve.wait_ge(act_sem, k0 + NCOPY)
                chain(dve.tensor_tensor(out=m[p][3][:, :],
                                        in0=cp[(k0 + 3) % NSLOT][:, :],
                                        in1=cp[(k0 + 4) % NSLOT][:, :], op=MAX))
                dve.wait_ge(dch_sem, di)
                chain(dve.tensor_tensor(out=f[p][0][:, :], in0=m[p][0][:, :],
                                        in1=m[p][1][:, :], op=MAX))
                dve.wait_ge(dch_sem, di)
                chain(dve.tensor_tensor(out=f[p][1][:, :], in0=m[p][2][:, :],
                                        in1=m[p][3][:, :], op=MAX))
                dve.wait_ge(dch_sem, di)
                chain(dve.tensor_tensor(out=n1[p][:, :], in0=f[p][0][:, :],
                                        in1=f[p][1][:, :], op=MAX))
                dve.wait_ge(dch_sem, di)
                chain(dve.tensor_tensor(out=n2[p][:, :], in0=n1[p][:, 0:512],
                                        in1=n1[p][:, 512:1024], op=MAX))
                dve.wait_ge(dch_sem, di)
                chain(dve.tensor_tensor(out=n3[p][:, :], in0=n2[p][:, 0:256],
                                        in1=n2[p][:, 256:512], op=MAX))
                dve.wait_ge(dch_sem, di)
                chain(
                    dve.tensor_reduce(
                        out=acc_sb[:, t : t + 1], in_=n3[p][:, :],
                        axis=mybir.AxisListType.X, op=MAX,
                    )
                )

    nc.compile()
    _NC[reps] = nc
    return nc


def _split3(v):
    """3-way fp8 split: v ~= a + b + c with ~2^-12 rel residual."""
    a = v.astype(F8)
    r = v - a.astype(np.float32)
    b = r.astype(F8)
    c = (r - b.astype(np.float32)).astype(F8)
    return a, b, c


def _prepare(previous_inclusion_score, nodes, adjacency_matrix, W_phi, W_theta):
    prev = np.asarray(previous_inclusion_score, dtype=np.float32)
    nodes = np.asarray(nodes, dtype=np.float32)
    adj = np.asarray(adjacency_matrix)
    W_phi = np.asarray(W_phi, dtype=np.float32)
    w = np.asarray(W_theta, dtype=np.float32)[:, 0]

    y = (nodes * w[None, :]).astype(np.float32)
    sq = np.sum(y * y, axis=1, dtype=np.float32)

    sqmax = float(sq.max())
    bound = 4.0 * sqmax + 8.0
    BIG0 = float(2.0 ** np.ceil(np.log2(bound)))   # > any possible d2
    s = BIGV / BIG0                                # psum scale, power of 2
    kq = int(np.ceil(np.log2(sqmax / 224.0))) if sqmax > 224.0 else 0

    adj_f8 = np.where(adj != 0, F8_ONE, np.uint8(0)).view(F8)

    lhs_rows = np.zeros((Y_ROWS, N), np.float32)
    rhs_rows = np.zeros((Y_ROWS, N), F8)
    r = 0
    for cdim in range(3):
        a, b, c = _split3(y[:, cdim])
        af, bf, cf = (x.astype(np.float32) for x in (a, b, c))
        for lv, rv in ((af, a), (af, b), (bf, a), (af, c), (cf, a), (bf, b)):
            lhs_rows[r] = -2.0 * s * lv
            rhs_rows[r] = rv
            r += 1
    q1, q2, q3 = _split3(sq * np.float32(2.0 ** -kq))
    for q in (q1, q2, q3):
        lhs_rows[r] = s * (2.0 ** kq)
        rhs_rows[r] = q
        r += 1
    assert r == Y_ROWS

    yz = np.zeros((128, 1, N), F8)
    yz[:Y_ROWS, 0, :] = rhs_rows

    eyeBIG = (np.eye(TSTEP, dtype=np.float32) * BIGV).astype(F8)
    lhs_f8 = lhs_rows.astype(F8)    # exact: power-of-2 scales of fp8 values

    in_maps = []
    for k in range(CORES):
        lhsT = np.zeros((128, TILES * 2, TSTEP), F8)
        for t in range(TILES):
            lo = k * ROWS + t * TSTEP
            adj_plane = 2 * t if t % 2 == 0 else 2 * t + 1
            y_plane = 2 * t + 1 if t % 2 == 0 else 2 * t
            lhsT[:, adj_plane, :] = eyeBIG
            lhsT[:Y_ROWS, y_plane, :] = lhs_f8[:, lo : lo + TSTEP]
        in_maps.append(
            {
                "adj": np.ascontiguousarray(adj_f8[k * ROWS : (k + 1) * ROWS]),
                "lhsT": lhsT,
                "yz": yz,
            }
        )
    return in_maps, prev, sq, BIG0, s, W_phi


def _finish(res, prev, sq, BIG0, s, W_phi):
    m = np.empty(N, dtype=np.float32)
    for k in range(CORES):
        gm = res.results[k]["gmax"]                  # [128, 8] (p, t)
        for t in range(TILES):
            lo = k * ROWS + t * TSTEP
            m[lo : lo + TSTEP] = gm[:, t]

    maxd2 = np.maximum(
        (m + np.float32(BIGV)) / np.float32(s) + (sq - np.float32(BIG0)), 0.0
    )
    max_dist = np.sqrt(maxd2)
    inc_mean = max_dist * np.float32(W_phi.mean())
    return ((prev + inc_mean) * 0.5).astype(np.float32)


def kernel(previous_inclusion_score, nodes, adjacency_matrix, W_phi, W_theta):
    in_maps, prev, sq, BIG0, s, W_phi = _prepare(
        previous_inclusion_score, nodes, adjacency_matrix, W_phi, W_theta
    )
    nc = _build_nc()
    res = run_bass_kernel_spmd(nc, in_maps, list(range(CORES)))
    return _finish(res, prev, sq, BIG0, s, W_phi)
